# revision 23
# baseline (speedup 1.0000x reference)
"""Trainium2 Bass kernel for nn_Bfly_BertIntermediate (butterfly MLP + bias + gelu).

Algorithm ("Monarch" factorization of the 10-layer butterfly over N=1024):
  layers 0..6  (strides 1..64) == block-diagonal A: 8 blocks of 128x128 per stack
  layers 7..9  (strides 128..512) == per-residue-128 mixing B: 8x8 over block idx j

Device pipeline per core (2048 tokens, data-parallel over 8 cores):
  x arrives feature-major f16 (host-side shard/layout prep) -> stage-A f16
  matmuls -> DVE cast-copy f32->f16 -> one batched SBUF->SBUF shuffle DMA per
  (stack, residue-group) -> stage-B flipped matmuls (token-major PSUM out, bias
  preloaded via K=1 ones x bias matmul) -> ScalarE gelu PSUM->SBUF ->
  SWDGE cast DMA (f16->f32) to HBM.

A/B/bias are a tiny host-side repacking of the twiddle weights (~0.1 GFLOP).
"""
import numpy as np

import concourse.bass as bass
import concourse.mybir as mybir
import concourse.tile as tile
from concourse import bacc, bass_utils

# problem shapes (hardcoded per harness contract)
B_, S_, N_ = 4, 4096, 1024
NSTACKS, LOG_N = 4, 10
SPLIT = 7                      # layers 0..6 -> A, 7..9 -> B
NJ = 8                         # 1024/128 blocks per stack
NG = 8                         # residue groups of 16
NCORES = 8
TOK = B_ * S_                  # 16384 tokens
TPC = TOK // NCORES            # 2048 tokens per core
ST_TOK = 512                   # supertile tokens
NSUP = TPC // ST_TOK           # 4 supertiles
NCH = ST_TOK // 128            # 4 chunks of 128 tokens

F32 = mybir.dt.float32
F16 = mybir.dt.float16

GELU = "gelu"


# ---------------------------------------------------------------- host factor
def _apply_layers(h, twiddle, layers):
    T, nstacks, n = h.shape
    for i in layers:
        stride = 1 << i
        nblk = n // (2 * stride)
        hr = h.reshape(T, nstacks, nblk, 2, stride)
        t = twiddle[:, i].reshape(nstacks, nblk, stride, 2, 2)
        hr = np.einsum('kbpoi,Tkbip->Tkbop', t, hr)
        h = hr.reshape(T, nstacks, n)
    return h


def _factor_weights(twiddle, bias):
    tw = np.asarray(twiddle, np.float64)
    eye = np.broadcast_to(np.eye(N_)[:, None, :], (N_, NSTACKS, N_)).copy()
    hA = _apply_layers(eye, tw, range(SPLIT))
    A_full = hA.transpose(1, 2, 0)          # [k, out_feat, in_feat]
    hB = _apply_layers(eye, tw, range(SPLIT, LOG_N))
    B_full = hB.transpose(1, 2, 0)

    # At[k, j, c, m] = A[k,j][m, c]  (lhsT layout: [K=c, M=m])
    At = np.empty((NSTACKS, NJ, 128, 128), np.float32)
    for j in range(NJ):
        blk = A_full[:, 128 * j:128 * (j + 1), 128 * j:128 * (j + 1)]
        At[:, j] = blk.transpose(0, 2, 1)

    # Bmat[k, r, j', j] = B_full[k, 128j'+r, 128j+r]
    jj = 128 * np.arange(NJ)
    Bmat = np.empty((NSTACKS, 128, NJ, NJ))
    for r in range(128):
        Bmat[:, r] = B_full[:, jj[:, None] + r, jj[None, :] + r]

    # Bw[k, g, q=(8a+j), of=(16j'+a)] = Bmat[k, 16g+a, j', j]
    # (q is a-major so the y shuffle's destination partitions are contiguous)
    Bw = np.zeros((NSTACKS, NG, 128, 128), np.float32)
    j16 = 16 * np.arange(NJ)
    j8 = np.arange(NJ)
    for k in range(NSTACKS):
        for g in range(NG):
            for a in range(16):
                Bw[k, g][np.ix_(8 * a + j8, j16 + a)] = Bmat[k, 16 * g + a].T

    # bias4[k, g, c*128 + 16j'+a] = bias[1024k + 128j' + 16g + a], repeated 4 c's
    bias = np.asarray(bias, np.float64)
    b1 = np.empty((NSTACKS, NG, 128))
    for k in range(NSTACKS):
        for g in range(NG):
            for jp in range(NJ):
                b1[k, g, 16 * jp:16 * jp + 16] = \
                    bias[1024 * k + 128 * jp + 16 * g + np.arange(16)]
    b4r = np.tile(b1[:, :, None, :], (1, 1, NCH, 1)).reshape(NSTACKS, NG * NCH * 128)
    return At, Bw.astype(np.float32), b4r.astype(np.float32)


# ---------------------------------------------------------------- device IR
def build_kernel():
    nc = bacc.Bacc()
    # x arrives already feature-major: xf[c, NJ*t + ...] -> [128, NJ*TPC]
    xf_d = nc.dram_tensor("xf", [128, NJ * TPC], F16, kind="ExternalInput")
    At_d = nc.dram_tensor("At", [NSTACKS, NJ, 128, 128], F16, kind="ExternalInput")
    Bw_d = nc.dram_tensor("Bw", [NSTACKS, NG, 128, 128], F16, kind="ExternalInput")
    b4_d = nc.dram_tensor("bias4", [NSTACKS, NG * NCH * 128], F16, kind="ExternalInput")
    out_d = nc.dram_tensor("out", [TPC, NSTACKS * N_], F32, kind="ExternalOutput")

    act_fn = (mybir.ActivationFunctionType.Gelu if GELU == "gelu"
              else mybir.ActivationFunctionType.Copy)

    BA_FREE = NJ * TPC   # y_ba free size
    SA_FREE = NG * TPC   # y_sa free size

    with tile.TileContext(nc) as tc:
        with (
            tc.tile_pool(name="consts", bufs=1) as consts,
            tc.tile_pool(name="xfa", bufs=1) as xfa_p,
            tc.tile_pool(name="yba", bufs=3) as yba_p,
            tc.tile_pool(name="ysa", bufs=2) as ysa_p,
            tc.tile_pool(name="outb", bufs=3) as outb_p,
            tc.tile_pool(name="ps_y", bufs=4, space="PSUM") as psy_p,
            tc.tile_pool(name="ps_o", bufs=4, space="PSUM") as pso_p,
        ):
            # e0: selector matrix, row 0 = ones.  e0.T @ r == broadcast of
            # r's partition-0 row to all output partitions, with a full
            # K=128 contraction (K=1 matmuls read as near-idle by the PE
            # activity monitor and keep the clock gate throttled).
            e0_sb = consts.tile([128, 128], F16)
            nc.vector.memset(e0_sb, 0.0)
            nc.vector.memset(e0_sb[0:1, :], 1.0)
            # b4full: bias rows live in partition 0 (one 512-wide block per
            # residue group), all other partitions zero.
            b4full = consts.tile([128, NG * NCH * 128], F16)
            nc.vector.memset(b4full, 0.0)

            # --- PE warmup: a dependency-free K=128 accumulation chain so
            # the HAM clock-gate sees a fully-busy window and lifts the PE
            # to 2.4 GHz before real work arrives.
            warm_ps = pso_p.tile([128, ST_TOK], F32, tag="pso")
            for w in range(16):
                nc.tensor.matmul(warm_ps[:, 0:256], e0_sb, b4full[:, 0:256],
                                 start=(w == 0), stop=(w == 15),
                                 skip_group_check=True)

            # --- persistent weights; At first (stage A gate), then x, then Bw
            At_sb = consts.tile([128, NSTACKS * NJ * 128], F16)  # part=c, free=(k,j,m)
            for ka in range(NSTACKS):
                nc.sync.dma_start(
                    out=At_sb[:, ka * NJ * 128:(ka + 1) * NJ * 128],
                    in_=bass.AP(tensor=At_d, offset=ka * NJ * 128 * 128,
                                ap=[[128, 128], [128 * 128, NJ], [1, 128]]))
            x_fa = xfa_p.tile([128, NJ * TPC], F16)
            for jh in range(NJ):  # load per-block so stage A starts early
                nc.sync.dma_start(
                    out=x_fa[:, TPC * jh:TPC * (jh + 1)],
                    in_=bass.AP(tensor=xf_d, offset=TPC * jh,
                                ap=[[NJ * TPC, 128], [1, TPC]]))
            Bw_sb = consts.tile([128, NSTACKS * NG * 128], F16)  # part=q, free=(k,g,of)
            nc.sync.dma_start(
                out=Bw_sb,
                in_=bass.AP(tensor=Bw_d, offset=0,
                            ap=[[128, 128], [128 * 128, NSTACKS * NG], [1, 128]]))

            # ---- per stack, full 2048-token stripes
            y_sa = {}
            for k in range(NSTACKS):
                nc.sync.dma_start(
                    out=b4full[0:1, :],
                    in_=bass.AP(tensor=b4_d, offset=k * NG * NCH * 128,
                                ap=[[1, 1], [1, NG * NCH * 128]]))

                # stage A, half-stripe (sp) outer so each half's shuffles can
                # start while the other half is still computing; y_ba tiles
                # are per-half so three halves pipeline through the pool
                HT = NSUP // 2 * ST_TOK  # 1024 tokens per half-stripe
                BAH = NJ * HT
                SAH = NG * HT
                for sp in range(2):
                    y_ba = yba_p.tile([128, BAH], F16, tag="yba")
                    ysa_tile = ysa_p.tile([128, SAH], F16, tag="ysa")
                    y_sa[sp] = ysa_tile
                    for j in range(NJ):
                        for ss in range(2):
                            st = 2 * sp + ss
                            yps = psy_p.tile([128, ST_TOK], F32)
                            nc.tensor.matmul(
                                yps,
                                At_sb[:, (k * NJ + j) * 128:(k * NJ + j + 1) * 128],
                                x_fa[:, TPC * j + ST_TOK * st:
                                     TPC * j + ST_TOK * (st + 1)],
                                start=True, stop=True)
                            nc.vector.tensor_copy(
                                y_ba[:, HT * j + ST_TOK * ss:
                                     HT * j + ST_TOK * (ss + 1)],
                                yps)

                    # shuffle: y_sa[8a+j, TPC*g+t] = y_ba[16g+a, HT*j+(t-HT*sp)]
                    # one 128-partition DMA per (group g, half sp): dst
                    # partitions q=8a+j enumerate contiguously over (a,j);
                    # src AP is partition-dim-first (a), then free (j, t).
                    for g in range(NG):
                        nc.gpsimd.dma_start(
                            out=bass.AP(tensor=y_sa[sp].tensor,
                                        offset=y_sa[sp].offset + HT * g,
                                        ap=[[SAH, 128], [1, HT]]),
                            in_=bass.AP(tensor=y_ba.tensor,
                                        offset=y_ba.offset + 16 * g * BAH,
                                        ap=[[BAH, 16], [HT, NJ], [1, HT]]))

                # stage B (flipped) + bias + gelu + store
                for st in range(NSUP):
                    out_sb = outb_p.tile([128, NCH * N_], F32, tag="outsb")
                    for g in range(NG):
                        pso = pso_p.tile([128, ST_TOK], F32, tag="pso")
                        nc.tensor.matmul(
                            pso, e0_sb,
                            b4full[:, g * ST_TOK:(g + 1) * ST_TOK],
                            start=True, stop=False, skip_group_check=True)
                        for c in range(NCH):
                            ysp = y_sa[st // 2]
                            nc.tensor.matmul(
                                pso[:, 128 * c:128 * (c + 1)],
                                ysp[:, HT * g + ST_TOK * (st % 2) + 128 * c:
                                    HT * g + ST_TOK * (st % 2) + 128 * (c + 1)],
                                Bw_sb[:, (k * NG + g) * 128:(k * NG + g + 1) * 128],
                                start=False, stop=(c == NCH - 1),
                                skip_group_check=True)
                        dst = bass.AP(
                            tensor=out_sb.tensor,
                            offset=out_sb.offset + 16 * g,
                            ap=[[NCH * N_, 128],   # partition (t)
                                [N_, NCH],         # c
                                [128, NJ],         # jp
                                [1, 16]])          # a
                        nc.scalar.activation(dst, pso, act_fn)
                    src = bass.AP(tensor=out_sb.tensor, offset=out_sb.offset,
                                  ap=[[NCH * N_, 128], [N_, NCH], [1, N_]])
                    dst = bass.AP(tensor=out_d,
                                  offset=st * ST_TOK * (NSTACKS * N_) + k * N_,
                                  ap=[[NSTACKS * N_, 128],
                                      [128 * NSTACKS * N_, NCH],
                                      [1, N_]])
                    nc.sync.dma_start(out=dst, in_=src)

    nc.finalize()
    return nc


_NC_CACHE = None


def kernel(hidden_states, twiddle, bias):
    global _NC_CACHE
    x = np.ascontiguousarray(np.asarray(hidden_states, np.float32)).reshape(TOK, N_)
    At, Bw, bias4 = _factor_weights(twiddle, bias)

    if _NC_CACHE is None:
        _NC_CACHE = build_kernel()
    nc = _NC_CACHE

    At16 = At.astype(np.float16)
    Bw16 = Bw.astype(np.float16)
    b416 = bias4.astype(np.float16)
    x16 = x.astype(np.float16)

    in_maps = []
    for i in range(NCORES):
        # feature-major per-core layout: xf[c, NJ*TPC] with xf[c, TPC*j + t]
        # = x[i*TPC + t, 128*j + c]
        xs = x16[i * TPC:(i + 1) * TPC].reshape(TPC, NJ, 128)
        xf = np.ascontiguousarray(xs.transpose(2, 1, 0)).reshape(128, NJ * TPC)
        in_maps.append({
            "xf": xf, "At": At16, "Bw": Bw16, "bias4": b416,
        })
    res = bass_utils.run_bass_kernel_spmd(nc, in_maps, core_ids=list(range(NCORES)))
    global LAST_RESULT
    LAST_RESULT = res
    out = np.concatenate([res.results[i]["out"] for i in range(NCORES)], axis=0)
    return out.reshape(B_, S_, NSTACKS * N_)


LAST_RESULT = None


if __name__ == "__main__":
    rng = np.random.default_rng(0)
    h = rng.standard_normal((B_, S_, N_), dtype=np.float32)
    tw = (rng.standard_normal((NSTACKS, LOG_N, N_ // 2, 2, 2)) * 2 ** -0.5).astype(np.float32)
    b = rng.standard_normal(NSTACKS * N_).astype(np.float32)
    out = kernel(h, tw, b)
    print("out", out.shape, out.dtype, np.abs(out).max())


# revision 24
# speedup vs baseline: 1.2354x; 1.2354x over previous
"""Trainium2 Bass kernel for nn_Bfly_BertIntermediate (butterfly MLP + bias + gelu).

Algorithm ("Monarch" factorization of the 10-layer butterfly over N=1024):
  layers 0..6  (strides 1..64) == block-diagonal A: 8 blocks of 128x128 per stack
  layers 7..9  (strides 128..512) == per-residue-128 mixing B: 8x8 over block idx j

Device pipeline per core (2048 tokens, data-parallel over 8 cores):
  x arrives feature-major f16 (host-side shard/layout prep) -> stage-A f16
  matmuls -> DVE cast-copy f32->f16 -> one batched SBUF->SBUF shuffle DMA per
  (stack, residue-group) -> stage-B flipped matmuls (token-major PSUM out, bias
  preloaded via K=1 ones x bias matmul) -> ScalarE gelu PSUM->SBUF ->
  SWDGE cast DMA (f16->f32) to HBM.

A/B/bias are a tiny host-side repacking of the twiddle weights (~0.1 GFLOP).
"""
import numpy as np

import concourse.bass as bass
import concourse.mybir as mybir
import concourse.tile as tile
from concourse import bacc, bass_utils

# problem shapes (hardcoded per harness contract)
B_, S_, N_ = 4, 4096, 1024
NSTACKS, LOG_N = 4, 10
SPLIT = 7                      # layers 0..6 -> A, 7..9 -> B
NJ = 8                         # 1024/128 blocks per stack
NG = 8                         # residue groups of 16
NCORES = 8
TOK = B_ * S_                  # 16384 tokens
TPC = TOK // NCORES            # 2048 tokens per core
ST_TOK = 512                   # supertile tokens
NSUP = TPC // ST_TOK           # 4 supertiles
NCH = ST_TOK // 128            # 4 chunks of 128 tokens

F32 = mybir.dt.float32
F16 = mybir.dt.float16

GELU = "gelu"


# ---------------------------------------------------------------- host factor
def _apply_layers(h, twiddle, layers):
    T, nstacks, n = h.shape
    for i in layers:
        stride = 1 << i
        nblk = n // (2 * stride)
        hr = h.reshape(T, nstacks, nblk, 2, stride)
        t = twiddle[:, i].reshape(nstacks, nblk, stride, 2, 2)
        hr = np.einsum('kbpoi,Tkbip->Tkbop', t, hr)
        h = hr.reshape(T, nstacks, n)
    return h


def _factor_weights(twiddle, bias):
    tw = np.asarray(twiddle, np.float64)
    eye = np.broadcast_to(np.eye(N_)[:, None, :], (N_, NSTACKS, N_)).copy()
    hA = _apply_layers(eye, tw, range(SPLIT))
    A_full = hA.transpose(1, 2, 0)          # [k, out_feat, in_feat]
    hB = _apply_layers(eye, tw, range(SPLIT, LOG_N))
    B_full = hB.transpose(1, 2, 0)

    # At[k, j, c, m] = A[k,j][m, c]  (lhsT layout: [K=c, M=m])
    At = np.empty((NSTACKS, NJ, 128, 128), np.float32)
    for j in range(NJ):
        blk = A_full[:, 128 * j:128 * (j + 1), 128 * j:128 * (j + 1)]
        At[:, j] = blk.transpose(0, 2, 1)

    # Bmat[k, r, j', j] = B_full[k, 128j'+r, 128j+r]
    jj = 128 * np.arange(NJ)
    Bmat = np.empty((NSTACKS, 128, NJ, NJ))
    for r in range(128):
        Bmat[:, r] = B_full[:, jj[:, None] + r, jj[None, :] + r]

    # Bw[k, g, q=(8a+j), of=(16j'+a)] = Bmat[k, 16g+a, j', j]
    # (q is a-major so the y shuffle's destination partitions are contiguous)
    Bw = np.zeros((NSTACKS, NG, 128, 128), np.float32)
    j16 = 16 * np.arange(NJ)
    j8 = np.arange(NJ)
    for k in range(NSTACKS):
        for g in range(NG):
            for a in range(16):
                Bw[k, g][np.ix_(8 * a + j8, j16 + a)] = Bmat[k, 16 * g + a].T

    # bias4[k, g, c*128 + 16j'+a] = bias[1024k + 128j' + 16g + a], repeated 4 c's
    bias = np.asarray(bias, np.float64)
    b1 = np.empty((NSTACKS, NG, 128))
    for k in range(NSTACKS):
        for g in range(NG):
            for jp in range(NJ):
                b1[k, g, 16 * jp:16 * jp + 16] = \
                    bias[1024 * k + 128 * jp + 16 * g + np.arange(16)]
    b4r = np.tile(b1[:, :, None, :], (1, 1, NCH, 1)).reshape(NSTACKS, NG * NCH * 128)
    return At, Bw.astype(np.float32), b4r.astype(np.float32)


# ---------------------------------------------------------------- device IR
def build_kernel():
    nc = bacc.Bacc()
    # x arrives already feature-major: xf[c, NJ*t + ...] -> [128, NJ*TPC]
    xf_d = nc.dram_tensor("xf", [128, NJ * TPC], F16, kind="ExternalInput")
    At_d = nc.dram_tensor("At", [NSTACKS, NJ, 128, 128], F16, kind="ExternalInput")
    Bw_d = nc.dram_tensor("Bw", [NSTACKS, NG, 128, 128], F16, kind="ExternalInput")
    b4_d = nc.dram_tensor("bias4", [NSTACKS, NG * NCH * 128], F16, kind="ExternalInput")
    out_d = nc.dram_tensor("out", [TPC, NSTACKS * N_], F32, kind="ExternalOutput")

    act_fn = (mybir.ActivationFunctionType.Gelu if GELU == "gelu"
              else mybir.ActivationFunctionType.Copy)

    BA_FREE = NJ * TPC   # y_ba free size
    SA_FREE = NG * TPC   # y_sa free size

    with tile.TileContext(nc) as tc:
        with (
            tc.tile_pool(name="consts", bufs=1) as consts,
            tc.tile_pool(name="xfa", bufs=1) as xfa_p,
            tc.tile_pool(name="yba", bufs=3) as yba_p,
            tc.tile_pool(name="ysa", bufs=3) as ysa_p,
            tc.tile_pool(name="outb", bufs=4) as outb_p,
            tc.tile_pool(name="ps_y", bufs=4, space="PSUM") as psy_p,
            tc.tile_pool(name="ps_o", bufs=4, space="PSUM") as pso_p,
        ):
            # e0: selector matrix, row 0 = ones.  e0.T @ r == broadcast of
            # r's partition-0 row to all output partitions, with a full
            # K=128 contraction (K=1 matmuls read as near-idle by the PE
            # activity monitor and keep the clock gate throttled).
            e0_sb = consts.tile([128, 128], F16)
            nc.vector.memset(e0_sb, 0.0)
            nc.vector.memset(e0_sb[0:1, :], 1.0)
            # b4full: bias rows live in partition 0 (one 512-wide block per
            # residue group), all other partitions zero.
            b4full = consts.tile([128, NG * NCH * 128], F16)
            nc.vector.memset(b4full, 0.0)

            # --- PE warmup: a dependency-free K=128 accumulation chain so
            # the HAM clock-gate sees a fully-busy window and lifts the PE
            # to 2.4 GHz before real work arrives.
            warm_ps = pso_p.tile([128, ST_TOK], F32, tag="pso")
            for w in range(16):
                nc.tensor.matmul(warm_ps[:, 0:256], e0_sb, b4full[:, 0:256],
                                 start=(w == 0), stop=(w == 15),
                                 skip_group_check=True)

            # --- persistent weights; At first (stage A gate), then x, then Bw
            At_sb = consts.tile([128, NSTACKS * NJ * 128], F16)  # part=c, free=(k,j,m)
            for ka in range(NSTACKS):
                nc.sync.dma_start(
                    out=At_sb[:, ka * NJ * 128:(ka + 1) * NJ * 128],
                    in_=bass.AP(tensor=At_d, offset=ka * NJ * 128 * 128,
                                ap=[[128, 128], [128 * 128, NJ], [1, 128]]))
            x_fa = xfa_p.tile([128, NJ * TPC], F16)
            for jh in range(NJ):  # load per-block so stage A starts early
                nc.sync.dma_start(
                    out=x_fa[:, TPC * jh:TPC * (jh + 1)],
                    in_=bass.AP(tensor=xf_d, offset=TPC * jh,
                                ap=[[NJ * TPC, 128], [1, TPC]]))
            Bw_sb = consts.tile([128, NSTACKS * NG * 128], F16)  # part=q, free=(k,g,of)
            nc.sync.dma_start(
                out=Bw_sb,
                in_=bass.AP(tensor=Bw_d, offset=0,
                            ap=[[128, 128], [128 * 128, NSTACKS * NG], [1, 128]]))

            # ---- per stack, full 2048-token stripes
            y_sa = {}
            for k in range(NSTACKS):
                nc.sync.dma_start(
                    out=b4full[0:1, :],
                    in_=bass.AP(tensor=b4_d, offset=k * NG * NCH * 128,
                                ap=[[1, 1], [1, NG * NCH * 128]]))

                # stage A, half-stripe (sp) outer so each half's shuffles can
                # start while the other half is still computing; y_ba tiles
                # are per-half so three halves pipeline through the pool
                HT = NSUP // 2 * ST_TOK  # 1024 tokens per half-stripe
                BAH = NJ * HT
                SAH = NG * HT
                for sp in range(2):
                    y_ba = yba_p.tile([128, BAH], F16, tag="yba")
                    ysa_tile = ysa_p.tile([128, SAH], F16, tag="ysa")
                    y_sa[sp] = ysa_tile
                    for j in range(NJ):
                        for ss in range(2):
                            st = 2 * sp + ss
                            yps = psy_p.tile([128, ST_TOK], F32)
                            nc.tensor.matmul(
                                yps,
                                At_sb[:, (k * NJ + j) * 128:(k * NJ + j + 1) * 128],
                                x_fa[:, TPC * j + ST_TOK * st:
                                     TPC * j + ST_TOK * (st + 1)],
                                start=True, stop=True)
                            nc.vector.tensor_copy(
                                y_ba[:, HT * j + ST_TOK * ss:
                                     HT * j + ST_TOK * (ss + 1)],
                                yps)

                    # shuffle: y_sa[8a+j, TPC*g+t] = y_ba[16g+a, HT*j+(t-HT*sp)]
                    # one 128-partition DMA per (group g, half sp): dst
                    # partitions q=8a+j enumerate contiguously over (a,j);
                    # src AP is partition-dim-first (a), then free (j, t).
                    for g in range(NG):
                        eng = nc.sync if g % 2 == 0 else nc.gpsimd
                        eng.dma_start(
                            out=bass.AP(tensor=y_sa[sp].tensor,
                                        offset=y_sa[sp].offset + HT * g,
                                        ap=[[SAH, 128], [1, HT]]),
                            in_=bass.AP(tensor=y_ba.tensor,
                                        offset=y_ba.offset + 16 * g * BAH,
                                        ap=[[BAH, 16], [HT, NJ], [1, HT]]))

                # stage B (flipped) + bias + gelu + store
                for st in range(NSUP):
                    out_sb = outb_p.tile([128, NCH * N_], F16, tag="outsb")
                    for g in range(NG):
                        pso = pso_p.tile([128, ST_TOK], F32, tag="pso")
                        nc.tensor.matmul(
                            pso, e0_sb,
                            b4full[:, g * ST_TOK:(g + 1) * ST_TOK],
                            start=True, stop=False, skip_group_check=True)
                        for c in range(NCH):
                            ysp = y_sa[st // 2]
                            nc.tensor.matmul(
                                pso[:, 128 * c:128 * (c + 1)],
                                ysp[:, HT * g + ST_TOK * (st % 2) + 128 * c:
                                    HT * g + ST_TOK * (st % 2) + 128 * (c + 1)],
                                Bw_sb[:, (k * NG + g) * 128:(k * NG + g + 1) * 128],
                                start=False, stop=(c == NCH - 1),
                                skip_group_check=True)
                        dst = bass.AP(
                            tensor=out_sb.tensor,
                            offset=out_sb.offset + 16 * g,
                            ap=[[NCH * N_, 128],   # partition (t)
                                [N_, NCH],         # c
                                [128, NJ],         # jp
                                [1, 16]])          # a
                        nc.scalar.activation(dst, pso, act_fn)
                    src = bass.AP(tensor=out_sb.tensor, offset=out_sb.offset,
                                  ap=[[NCH * N_, 128], [N_, NCH], [1, N_]])
                    dst = bass.AP(tensor=out_d,
                                  offset=st * ST_TOK * (NSTACKS * N_) + k * N_,
                                  ap=[[NSTACKS * N_, 128],
                                      [128 * NSTACKS * N_, NCH],
                                      [1, N_]])
                    nc.gpsimd.dma_start(out=dst, in_=src)

    nc.finalize()
    return nc


_NC_CACHE = None


def kernel(hidden_states, twiddle, bias):
    global _NC_CACHE
    x = np.ascontiguousarray(np.asarray(hidden_states, np.float32)).reshape(TOK, N_)
    At, Bw, bias4 = _factor_weights(twiddle, bias)

    if _NC_CACHE is None:
        _NC_CACHE = build_kernel()
    nc = _NC_CACHE

    At16 = At.astype(np.float16)
    Bw16 = Bw.astype(np.float16)
    b416 = bias4.astype(np.float16)
    x16 = x.astype(np.float16)

    in_maps = []
    for i in range(NCORES):
        # feature-major per-core layout: xf[c, NJ*TPC] with xf[c, TPC*j + t]
        # = x[i*TPC + t, 128*j + c]
        xs = x16[i * TPC:(i + 1) * TPC].reshape(TPC, NJ, 128)
        xf = np.ascontiguousarray(xs.transpose(2, 1, 0)).reshape(128, NJ * TPC)
        in_maps.append({
            "xf": xf, "At": At16, "Bw": Bw16, "bias4": b416,
        })
    res = bass_utils.run_bass_kernel_spmd(nc, in_maps, core_ids=list(range(NCORES)))
    global LAST_RESULT
    LAST_RESULT = res
    out = np.concatenate([res.results[i]["out"] for i in range(NCORES)], axis=0)
    return out.reshape(B_, S_, NSTACKS * N_)


LAST_RESULT = None


if __name__ == "__main__":
    rng = np.random.default_rng(0)
    h = rng.standard_normal((B_, S_, N_), dtype=np.float32)
    tw = (rng.standard_normal((NSTACKS, LOG_N, N_ // 2, 2, 2)) * 2 ** -0.5).astype(np.float32)
    b = rng.standard_normal(NSTACKS * N_).astype(np.float32)
    out = kernel(h, tw, b)
    print("out", out.shape, out.dtype, np.abs(out).max())
